# revision 1
# baseline (speedup 1.0000x reference)
# Deformable Conv2d (B=4, C=CO=64, H=W=192, K=3, pad=1) on 8 Trainium2 NeuronCores.
#
# Strategy (data-parallel over B x half-image, 8 shards):
#   out[o,px] = sum_k sum_{c} wk[o,c,k] * bilinear_sample(x, px + base_k + (dy,dx)_k)
# Bilinear sampling with |d|<T/2 is an exact T-tap separable "hat filter" over a
# FIXED stencil around the base tap:  w(j) = relu(1 - |d - j|).  So
#   out[o,px] = sum_k sum_{u,v in taps} wy_ku[px]*wx_kv[px] * r_k[o, px+(by+u, bx+v)]
# where r_k = W_k @ x are 1x1-conv response planes.  On-device:
#   - PE computes, per 128-pixel tile (2 rows x 64 cols), all shifted r-slices
#     group-by-(sy,sx) as data-stationary matmuls (M=128: both rows in one
#     matmul via a 2D free-dim AP on the x-slab slice). Out -> PSUM.
#   - The offset conv (18 channels) is 9 matmuls per row-pair -> PSUM.
#   - ACT builds the hat weights from the offsets.
#   - The 81 modulated-accumulation terms (the old DVE-only bottleneck:
#     81 x 192ns scalar_tensor_tensor = 15.5us/tile) are now split across
#     three engines:
#       * 'wide' groups: ACT copies the whole PSUM group to SBUF bf16 once
#         (wide, cheap/elem), then DVE (127ns) and GPSIMD (184ns) run
#         scalar_tensor_tensor FMAs from SBUF.
#       * remaining terms: DVE FMAs straight from PSUM (192ns) or
#         ACT per-term products (238ns) summed by GPSIMD adds (222ns).
#   - Output DMAs out flat; the host wrapper restores [B, CO, H, W] layout.
import os
import numpy as np

B, C, CO, H, W = 4, 64, 64, 192, 192
K, PAD, KK = 3, 1, 9
N_CORES = 8
HALVES = N_CORES // B            # 2 half-images per batch sample
ROWS = H // HALVES               # 96 rows per core
HALO = 3                         # row halo each side (covers 5-tap reach)
PADC = 3                         # col zero-pad each side
WP = W + 2 * PADC                # 198
RSLAB = ROWS + 2 * HALO          # 102
TAPS = 3
LOOPT = int(os.environ.get("DFC_LOOPT", "1"))    # hw-loop trip count (timing)
TR = (TAPS - 1) // 2
CB = 3                           # col blocks of 64 per row-pair
TILE_ROWS = ROWS // 2            # 48 row-pairs
N_TILES = TILE_ROWS * CB         # 144

# engine-split knobs (tuned via TimelineSim sweeps)
N_WIDE = int(os.environ.get("DFC_WIDE", "15"))   # largest-nk groups wide-copied
DW = int(os.environ.get("DFC_DW", "34"))         # DVE stt share of wide terms
PW = int(os.environ.get("DFC_PW", "29"))         # DVE ts-product share (GPS adds)
AP_TERMS = int(os.environ.get("DFC_AP", "8"))    # ACT-product terms (GPS adds)
NACC_V = 3                                       # DVE accumulator chains
SUPER = 8                                        # products per GPSIMD super-add
PSUM_BUFS = int(os.environ.get("DFC_PSUM_BUFS", "6"))

# channel permutation of the offset conv (faithful to reference's reshape/
# transpose dance): new ch j<9 -> dy_j, j>=9 -> dx_{j-9}
DYPERM = [0, 4, 8, 12, 16, 3, 7, 11, 15]
DXPERM = [2, 6, 10, 14, 1, 5, 9, 13, 17]

BASE = [(k // 3 - 1, k % 3 - 1) for k in range(KK)]  # (by, bx) per k

# (sy, sx) groups: absolute shifts, with the k's whose hat window contains them
SHIFTS = []
for sy in range(-1 - TR, 2 + TR):
    for sx in range(-1 - TR, 2 + TR):
        ks = [k for k in range(KK)
              if abs(sy - BASE[k][0]) <= TR and abs(sx - BASE[k][1]) <= TR]
        if ks:
            SHIFTS.append((sy, sx, ks))
MAX_GROUP_K = 8  # keep matmul N = nk*64 <= 512 (one PSUM bank)

GROUPS = []
for sy, sx, ks in SHIFTS:
    for i in range(0, len(ks), MAX_GROUP_K):
        GROUPS.append((sy, sx, ks[i:i + MAX_GROUP_K]))
WTOT = sum(len(ks) for _, _, ks in GROUPS) * CO


def _term_assignment():
    """Static per-term engine plan.

    Returns (wide_set, plan) where plan[(gi, j)] in {'VW','PW','V','A'}:
      VW: DVE scalar_tensor_tensor FMA from the wide-copied SBUF group
      PW: DVE tensor_scalar product (bf16, 4x) into a super tile;
          GPSIMD sums super tiles with wide tensor_tensor adds
      V:  DVE scalar_tensor_tensor FMA straight from PSUM
      A:  ACT product (PSUM->SBUF slot of a super tile) + GPSIMD super-add
    """
    order = sorted(range(len(GROUPS)), key=lambda g: -len(GROUPS[g][2]))
    wide = set(order[:N_WIDE])
    wide_terms = [(g, j) for g in range(len(GROUPS)) if g in wide
                  for j in range(len(GROUPS[g][2]))]
    rest_terms = [(g, j) for g in range(len(GROUPS)) if g not in wide
                  for j in range(len(GROUPS[g][2]))]
    plan = {}
    # interleave stt-FMA vs product assignment over the wide terms
    nw = len(wide_terms)
    dw = min(DW, nw)
    stride = nw / max(dw, 1)
    vw_idx = {int(i * stride) for i in range(dw)}
    pw_left = PW
    for i, t in enumerate(wide_terms):
        if i in vw_idx:
            plan[t] = 'VW'
        elif pw_left > 0:
            plan[t] = 'PW'
            pw_left -= 1
        else:
            plan[t] = 'VW'
    nr = len(rest_terms)
    ap = min(AP_TERMS, nr)
    stride = nr / max(ap, 1)
    a_idx = {int(i * stride) for i in range(ap)}
    for i, t in enumerate(rest_terms):
        plan[t] = 'A' if i in a_idx else 'V'
    return wide, plan


_CACHE = {}


def _build_program():
    import concourse.bacc as bacc
    import concourse.mybir as mybir
    from concourse import tile

    f32 = mybir.dt.float32
    bf16 = mybir.dt.bfloat16
    MUL = mybir.AluOpType.mult
    ADD = mybir.AluOpType.add
    AF = mybir.ActivationFunctionType

    wide, plan = _term_assignment()

    nc = bacc.Bacc("TRN2", num_devices=N_CORES)
    xslab_d = nc.dram_tensor("xslab", [C + 1, RSLAB, WP], bf16, kind="ExternalInput")
    woffb_d = nc.dram_tensor("woffb", [C + 1, KK * 2 * KK], bf16, kind="ExternalInput")
    wstack_d = nc.dram_tensor("wstack", [C, WTOT], bf16, kind="ExternalInput")
    out_d = nc.dram_tensor("out", [N_TILES * 128, CO], f32, kind="ExternalOutput")

    NW = KK * TAPS  # columns of WY / WX

    with tile.TileContext(nc) as tc:
        with (
            tc.tile_pool(name="slab", bufs=1) as slab_pool,
            tc.tile_pool(name="consts", bufs=1) as const_pool,
            tc.tile_pool(name="wts", bufs=3) as wts_pool,
            tc.tile_pool(name="acc", bufs=3) as acc_pool,
            tc.tile_pool(name="prod", bufs=6) as prod_pool,
            tc.tile_pool(name="wcp", bufs=4) as wcp_pool,
            tc.tile_pool(name="psum", bufs=PSUM_BUFS, space="PSUM") as psum_pool,
            tc.tile_pool(name="ppsum", bufs=2, space="PSUM") as ppsum_pool,
        ):
            # per-partition constants for activation bias operands
            cvals = sorted({float(-(ui - TR)) for ui in range(TAPS)} | {1.0})
            cmap = {}
            for ci, v in enumerate(cvals):
                ct = const_pool.tile([128, 1], f32, tag=f"c{ci}",
                                     name=f"const{ci}")
                nc.vector.memset(ct[:, :], v)
                cmap[v] = ct

            zsup = const_pool.tile([128, SUPER * CO], bf16, tag="zsup",
                                   name="zsup")
            nc.vector.memset(zsup[:, :], 0.0)

            xsb = slab_pool.tile([C + 1, RSLAB, WP], bf16)
            nc.sync.dma_start(xsb[:, :, :], xslab_d.ap()[:, :, :])
            woffb = const_pool.tile([C + 1, KK * 2 * KK], bf16)
            nc.sync.dma_start(woffb[:, :], woffb_d.ap()[:, :])
            wstack = const_pool.tile([C, WTOT], bf16)
            nc.sync.dma_start(wstack[:, :], wstack_d.ap()[:, :])

            import contextlib
            loop_cm = tc.For_i(0, LOOPT, 1) if LOOPT > 1 else contextlib.nullcontext()
            with loop_cm:
              for hh in range(TILE_ROWS):
                r0 = 2 * hh + HALO              # slab row of row-pair's first row

                # ---- offset conv for the whole row-pair: p[px, 3*18] ----
                p_ps = ppsum_pool.tile([128, CB * 2 * KK], f32, tag="p")
                for cb in range(CB):
                    c0 = PADC + cb * 64
                    for row in range(2):
                        for k in range(KK):
                            by, bx = BASE[k]
                            nc.tensor.matmul(
                                p_ps[row * 64:(row + 1) * 64,
                                     cb * 18:(cb + 1) * 18],
                                xsb[0:C + 1, r0 + row + by,
                                    c0 + bx:c0 + bx + 64],
                                woffb[:, k * 18:(k + 1) * 18],
                                start=(k == 0), stop=(k == KK - 1),
                            )

                # ---- hat weights batched: WY/WX [128, (cb, k, u)] ----
                wy = wts_pool.tile([128, CB * NW], f32, tag="wy")
                wx = wts_pool.tile([128, CB * NW], f32, tag="wx")
                tmp = wts_pool.tile([128, CB * KK], f32, tag="tmp")
                for ax, wt in ((0, wy), (1, wx)):
                    for ui in range(TAPS):
                        u = ui - TR
                        nc.scalar.activation(
                            tmp[:, :],
                            p_ps.rearrange("p (c t) -> p c t", t=2 * KK)
                                [:, :, ax * KK:(ax + 1) * KK],
                            AF.Abs, bias=cmap[float(-u)][:, :])
                        nc.scalar.activation(
                            wt.rearrange("p (c k t) -> p c k t", t=TAPS, k=KK)
                              [:, :, :, ui],
                            tmp[:, :], AF.Relu,
                            scale=-1.0, bias=cmap[1.0][:, :])
                # products: WYX col = cb*9T^2 + k*T^2 + ui*T + vi
                TT2 = TAPS * TAPS
                wyx = wts_pool.tile([128, CB * KK * TT2], f32, tag="wyx")
                for ui in range(TAPS):
                    for vi in range(TAPS):
                        nc.vector.tensor_tensor(
                            wyx.rearrange("p (c k t) -> p c k t", t=TT2, k=KK)
                               [:, :, :, ui * TAPS + vi],
                            wy.rearrange("p (c k t) -> p c k t", t=TAPS, k=KK)
                              [:, :, :, ui],
                            wx.rearrange("p (c k t) -> p c k t", t=TAPS, k=KK)
                              [:, :, :, vi],
                            MUL)

                for cb in range(CB):
                    t_idx = hh * CB + cb
                    c0 = PADC + cb * 64         # slab col of tile's first col

                    # DVE stt chains + GPSIMD super-add accumulator
                    vacc = [acc_pool.tile([128, CO], f32, tag=f"va{a}",
                                          name=f"va{a}_{t_idx}")
                            for a in range(NACC_V)]
                    gacc2 = acc_pool.tile([128, SUPER * CO], f32, tag="gacc2",
                                          name=f"gacc2_{t_idx}")
                    v_started = [False] * NACC_V
                    vterm = [0]
                    state = {"g_started": False, "slot": 0, "sup": None}

                    def sc_of(k, sy, sx):
                        ui = sy - BASE[k][0] + TR
                        vi = sx - BASE[k][1] + TR
                        col = cb * KK * TT2 + k * TT2 + ui * TAPS + vi
                        return wyx[:, col:col + 1]

                    def dve_fma(rsl, sc):
                        a = vterm[0] % NACC_V
                        vterm[0] += 1
                        if not v_started[a]:
                            nc.vector.tensor_scalar_mul(vacc[a][:, :], rsl, sc)
                            v_started[a] = True
                        else:
                            nc.vector.scalar_tensor_tensor(
                                vacc[a][:, :], rsl, sc, vacc[a][:, :], MUL, ADD)

                    def super_slot():
                        # returns (tile, slot); a full tile goes to GPSIMD
                        if state["sup"] is None:
                            state["nsup"] = state.get("nsup", 0) + 1
                            state["sup"] = prod_pool.tile(
                                [128, SUPER * CO], bf16, tag="sup",
                                name=f"sup_{t_idx}_{state['nsup']}")
                            state["slot"] = 0
                        s = state["slot"]
                        state["slot"] += 1
                        return state["sup"], s

                    def flush_super():
                        sup, n = state["sup"], state["slot"]
                        if sup is None or n == 0:
                            return
                        fd = n * CO
                        full = SUPER * CO
                        if not state["g_started"]:
                            nc.gpsimd.tensor_tensor(
                                gacc2[:, 0:fd], sup[:, 0:fd],
                                zsup[:, 0:fd], ADD)
                            if fd < full:
                                nc.gpsimd.tensor_tensor(
                                    gacc2[:, fd:full], zsup[:, fd:full],
                                    zsup[:, fd:full], ADD)
                            state["g_started"] = True
                        else:
                            nc.gpsimd.tensor_tensor(
                                gacc2[:, 0:fd], gacc2[:, 0:fd],
                                sup[:, 0:fd], ADD)
                        state["sup"] = None
                        state["slot"] = 0

                    def add_product(write_product):
                        sup, s = super_slot()
                        write_product(sup[:, s * CO:(s + 1) * CO])
                        if state["slot"] == SUPER:
                            flush_super()

                    woff = 0
                    for gi, (sy, sx, ks) in enumerate(GROUPS):
                        nk = len(ks)
                        r_ps = psum_pool.tile([128, nk * CO], f32, tag="r")
                        for row in range(2):
                            nc.tensor.matmul(
                                r_ps[row * 64:(row + 1) * 64, :],
                                xsb[0:C, r0 + row + sy,
                                    c0 + sx:c0 + sx + 64],
                                wstack[:, woff:woff + nk * CO],
                                start=True, stop=True)
                        woff += nk * CO

                        if gi in wide:
                            rcp = wcp_pool.tile([128, MAX_GROUP_K * CO], bf16,
                                                tag="rcp")
                            nc.scalar.activation(rcp[:, 0:nk * CO], r_ps[:, :],
                                                 AF.Copy)
                            for j, k in enumerate(ks):
                                sc = sc_of(k, sy, sx)
                                rsl = rcp[:, j * CO:(j + 1) * CO]
                                if plan[(gi, j)] == 'VW':
                                    dve_fma(rsl, sc)
                                else:   # 'PW': DVE bf16 product, GPS adds
                                    add_product(
                                        lambda dst, rsl=rsl, sc=sc:
                                        nc.vector.tensor_scalar_mul(
                                            dst, rsl, sc))
                        else:
                            for j, k in enumerate(ks):
                                sc = sc_of(k, sy, sx)
                                rsl = r_ps[:, j * CO:(j + 1) * CO]
                                if plan[(gi, j)] == 'V':
                                    dve_fma(rsl, sc)
                                else:   # 'A': ACT product, GPS adds
                                    add_product(
                                        lambda dst, rsl=rsl, sc=sc:
                                        nc.scalar.activation(
                                            dst, rsl, AF.Copy, scale=sc))

                    flush_super()

                    # ---- fold gacc2 [SUPER*CO] -> [CO] on DVE (tree) ----
                    w = SUPER * CO
                    while w > CO:
                        h = w // 2
                        nc.vector.tensor_tensor(gacc2[:, 0:h], gacc2[:, 0:h],
                                                gacc2[:, h:w], ADD)
                        w = h

                    # ---- combine chains -> osum -> DMA ----
                    osum = acc_pool.tile([128, CO], f32, tag="osum")
                    nc.vector.tensor_tensor(vacc[0][:, :], vacc[0][:, :],
                                            vacc[2][:, :], ADD)
                    nc.gpsimd.tensor_tensor(vacc[1][:, :], vacc[1][:, :],
                                            gacc2[:, 0:CO], ADD)
                    nc.gpsimd.tensor_tensor(osum[:, :], vacc[0][:, :],
                                            vacc[1][:, :], ADD)
                    nc.sync.dma_start(
                        out_d.ap()[t_idx * 128:(t_idx + 1) * 128, :], osum[:, :])

    nc.compile()
    return nc


def _prep_weights(w_deform, w_offset, b_offset):
    # offset conv weights with output channels permuted to [dy(9), dx(9)],
    # bias folded as contract-row 64 on the center (k==4) tap block.
    perm = DYPERM + DXPERM
    wo = w_offset[perm]          # [18, C, 3, 3]
    bo = b_offset[perm]          # [18]
    woffb = np.zeros((C + 1, KK * 18), np.float32)
    for k in range(KK):
        ky, kx = k // 3, k % 3
        woffb[:C, k * 18:(k + 1) * 18] = wo[:, :, ky, kx].T
    woffb[C, 4 * 18:5 * 18] = bo
    # stacked deform weights per (sy,sx) group
    blocks = []
    for sy, sx, ks in GROUPS:
        for k in ks:
            blocks.append(w_deform[:, :, k // 3, k % 3].T)  # [C, CO]
    wstack = np.concatenate(blocks, axis=1).astype(np.float32)
    import ml_dtypes
    return woffb.astype(ml_dtypes.bfloat16), wstack.astype(ml_dtypes.bfloat16)


def kernel(x, w_deform, w_offset, b_offset):
    from concourse.bass_utils import run_bass_kernel_spmd

    if "nc" not in _CACHE:
        _CACHE["nc"] = _build_program()
    nc = _CACHE["nc"]

    woffb, wstack = _prep_weights(
        np.asarray(w_deform, np.float32),
        np.asarray(w_offset, np.float32),
        np.asarray(b_offset, np.float32))

    x = np.asarray(x, np.float32)
    in_maps = []
    for core in range(N_CORES):
        b, half = core // HALVES, core % HALVES
        import ml_dtypes
        slab = np.zeros((C + 1, RSLAB, WP), ml_dtypes.bfloat16)
        slab[C] = 1.0
        r_lo = half * ROWS - HALO
        r_hi = half * ROWS + ROWS + HALO
        src_lo, src_hi = max(r_lo, 0), min(r_hi, H)
        slab[:C, src_lo - r_lo:src_hi - r_lo, PADC:PADC + W] = \
            x[b, :, src_lo:src_hi, :].astype(ml_dtypes.bfloat16)
        in_maps.append({"xslab": slab, "woffb": woffb, "wstack": wstack})

    res = run_bass_kernel_spmd(nc, in_maps, core_ids=list(range(N_CORES)))

    out = np.empty((B, CO, H, W), np.float32)
    for core in range(N_CORES):
        b, half = core // HALVES, core % HALVES
        o = res.results[core]["out"]          # [144*128, 64]
        o = o.reshape(TILE_ROWS, CB, 2, 64, CO)
        o = o.transpose(4, 0, 2, 1, 3).reshape(CO, ROWS, W)
        out[b, :, half * ROWS:(half + 1) * ROWS, :] = o
    return out


if __name__ == "__main__":
    xs = {k: np.load(f"/tmp/in_{k}.npy") for k in
          ("x", "w_deform", "w_offset", "b_offset")}
    got = kernel(**xs)
    exp = np.load("/tmp/expected.npy")
    err = np.abs(got - exp)
    rel = np.linalg.norm(got - exp) / np.linalg.norm(exp)
    print(f"absmax={err.max():.6f} rel-l2={rel:.3e}")



# revision 3
# speedup vs baseline: 4.0342x; 4.0342x over previous
# Deformable Conv2d (B=4, C=CO=64, H=W=192, K=3, pad=1) on 8 Trainium2 cores.
#
# v2: wide-op redesign of the modulated accumulation (vs the per-term
# scalar_tensor_tensor baseline).  Math: with bilinear offsets |d|<1.5,
#   out[px,o] = sum_{t=(k,u,v)} wy[px,k,u]*wx[px,k,v] * (W_k @ x_s(t))[px,o]
# where s(t) = base_k + (u,v) is one of 25 absolute shifts.  Per 128-pixel
# tile (2 rows x 64 cols):
#   - PE: 26 shift-group matmuls with M=128 (both rows in one matmul via a
#     2D stationary AP), into 4 PSUM chunks of [24,24,24,9] terms.
#   - ACT: one wide PSUM->SBUF bf16 copy per chunk (+ hat-weight builds).
#   - DVE: hat weights replicated x64 into wrep[128, 81*64] bf16 once per
#     row-pair (46 strided seed copies + 6 log2 doublings in 4x mode), then
#     4 wide tensor_tensor products (2x bf16; TT is single-port so it never
#     contends with GpSimd), then a 7-level block tree-fold 81*64 -> 64.
#   - Pool: configurable fold levels (default the 1280/640-elem middle ones).
import os
import numpy as np

B, C, CO, H, W = 4, 64, 64, 192, 192
K, PAD, KK = 3, 1, 9
N_CORES = 8
HALVES = N_CORES // B            # 2 half-images per batch sample
ROWS = H // HALVES               # 96 rows per core
HALO = 3                         # row halo each side
PADC = 3                         # col zero-pad each side
WP = W + 2 * PADC                # 198
RSLAB = ROWS + 2 * HALO          # 102
TAPS = 3
TR = (TAPS - 1) // 2
CB = 3                           # col blocks of 64 per row-pair
TILE_ROWS = ROWS // 2            # 48 row-pairs
N_TILES = TILE_ROWS * CB         # 144
NT = 81                          # modulation terms per pixel
LOOPT = int(os.environ.get("DFC_LOOPT", "1"))

# fold levels run on GpSimd (comma list of level indices, level 0 = widest)
POOL_LEVELS = {int(x) for x in
               os.environ.get("DFC_POOL_LEVELS", "1,2").split(",") if x != ""}
WTS_BUFS = int(os.environ.get("DFC_WTS_BUFS", "2"))
WREP_BUFS = int(os.environ.get("DFC_WREP_BUFS", "2"))
RCP_BUFS = int(os.environ.get("DFC_RCP_BUFS", "3"))
PROD_BUFS = int(os.environ.get("DFC_PROD_BUFS", "2"))
PSUM_BUFS = int(os.environ.get("DFC_PSUM_BUFS", "2"))
PPSUM_BUFS = int(os.environ.get("DFC_PPSUM_BUFS", "2"))
# parallel fold trees: Pool reduces the last POOL_SPLIT blocks while DVE
# reduces the rest; 0 disables (POOL_LEVELS ping-pong mode instead)
POOL_SPLIT = int(os.environ.get("DFC_POOL_SPLIT", "13"))
POOL_PROD = int(os.environ.get("DFC_POOL_PROD", "1"))   # last chunk's product on Pool
POOL_XTERMS = int(os.environ.get("DFC_POOL_XTERMS", "0"))  # extra product terms on Pool
OSUM_ACT = int(os.environ.get("DFC_OSUM_ACT", "1"))     # final copy on ACT
F32_TAIL = int(os.environ.get("DFC_F32_TAIL", "1"))     # finish fold in f32 on Pool
MERGE_RELU = int(os.environ.get("DFC_MERGE_RELU", "1"))  # 2 wide Relu vs 6
DMA_DOUBLE = int(os.environ.get("DFC_DMA_DOUBLE", "0"))  # widest wrep doubling on DMA
GROUP_SEED = int(os.environ.get("DFC_GROUP_SEED", "1"))  # per-group wy*wx -> wrep seed
PROD_SPLIT = int(os.environ.get("DFC_PROD_SPLIT", "0"))  # split products into <=N-term ops

DYPERM = [0, 4, 8, 12, 16, 3, 7, 11, 15]
DXPERM = [2, 6, 10, 14, 1, 5, 9, 13, 17]

BASE = [(k // 3 - 1, k % 3 - 1) for k in range(KK)]  # (by, bx) per k

SHIFTS = []
for sy in range(-1 - TR, 2 + TR):
    for sx in range(-1 - TR, 2 + TR):
        ks = [k for k in range(KK)
              if abs(sy - BASE[k][0]) <= TR and abs(sx - BASE[k][1]) <= TR]
        if ks:
            SHIFTS.append((sy, sx, ks))
MAX_GROUP_K = 8  # matmul N = nk*64 <= 512 (one PSUM bank)

GROUPS = []
for sy, sx, ks in SHIFTS:
    for i in range(0, len(ks), MAX_GROUP_K):
        GROUPS.append((sy, sx, ks[i:i + MAX_GROUP_K]))

CHUNK_MAX = int(os.environ.get("DFC_CHUNK_MAX", "24"))  # terms per PSUM chunk


def _pack_chunks():
    order = sorted(range(len(GROUPS)), key=lambda g: -len(GROUPS[g][2]))
    chunks, sizes = [], []
    for g in order:
        nk = len(GROUPS[g][2])
        for ci in range(len(chunks)):
            if sizes[ci] + nk <= CHUNK_MAX:
                chunks[ci].append(g)
                sizes[ci] += nk
                break
        else:
            chunks.append([g])
            sizes.append(nk)
    return chunks, sizes


CHUNKS, CHUNK_SIZES = _pack_chunks()
GORDER = [g for ch in CHUNKS for g in ch]
TERM_BASE = {}
_t = 0
for g in GORDER:
    TERM_BASE[g] = _t
    _t += len(GROUPS[g][2])
assert _t == NT
WTOT = NT * CO


def _group_grids():
    """Per-group (sy,sx): full (a,b) grids of its ks for the fused
    wy*wx -> wrep-seed tensor_tensor.  Ragged groups split by a-row."""
    grids = []
    for g in GORDER:
        sy, sx, ks = GROUPS[g]
        t0 = TERM_BASE[g]
        rows = {}
        for k in ks:
            rows.setdefault(k // 3, []).append(k % 3)
        avs = sorted(rows)
        full = (len({tuple(v) for v in rows.values()}) == 1 and
                avs == list(range(avs[0], avs[0] + len(avs))))
        if full:
            bs = rows[avs[0]]
            assert bs == list(range(bs[0], bs[0] + len(bs)))
            grids.append((sy, sx, avs[0], bs[0], len(avs), len(bs), t0))
        else:
            j = 0
            for a in avs:
                bs = rows[a]
                assert bs == list(range(bs[0], bs[0] + len(bs)))
                grids.append((sy, sx, a, bs[0], 1, len(bs), t0 + j))
                j += len(bs)
    return grids


GROUP_GRIDS = _group_grids()


def _seed_rows():
    """Seed-copy plan: per group, per k//3-row, a stride-8 source slice in
    k-major wyx (src col of term (k,u,v) is 9k+3u+v = 8k + (3sy+sx+8))."""
    rows = []
    for g in GORDER:
        sy, sx, ks = GROUPS[g]
        cc = 3 * sy + sx + 8
        t0 = TERM_BASE[g]
        j = 0
        from itertools import groupby
        for a, grp in groupby(ks, key=lambda k: k // 3):
            bs = [k % 3 for k in grp]
            assert bs == list(range(bs[0], bs[0] + len(bs)))
            src0 = 8 * (3 * a + bs[0]) + cc
            rows.append((src0, len(bs), t0 + j))
            j += len(bs)
    return rows


SEED_ROWS = _seed_rows()

_CACHE = {}


def _build_program():
    import concourse.bacc as bacc
    import concourse.mybir as mybir
    from concourse import tile

    f32 = mybir.dt.float32
    bf16 = mybir.dt.bfloat16
    MUL = mybir.AluOpType.mult
    ADD = mybir.AluOpType.add
    AF = mybir.ActivationFunctionType

    nc = bacc.Bacc("TRN2", num_devices=N_CORES)
    xslab_d = nc.dram_tensor("xslab", [C + 1, RSLAB, WP], bf16, kind="ExternalInput")
    woffb_d = nc.dram_tensor("woffb", [C + 1, KK * 2 * KK], bf16, kind="ExternalInput")
    wstack_d = nc.dram_tensor("wstack", [C, WTOT], bf16, kind="ExternalInput")
    out_d = nc.dram_tensor("out", [N_TILES * 128, CO], f32, kind="ExternalOutput")

    NW = KK * TAPS               # 27
    TT2 = TAPS * TAPS            # 9

    with tile.TileContext(nc) as tc:
        with (
            tc.tile_pool(name="slab", bufs=1) as slab_pool,
            tc.tile_pool(name="consts", bufs=1) as const_pool,
            tc.tile_pool(name="wts", bufs=WTS_BUFS) as wts_pool,
            tc.tile_pool(name="wrep", bufs=WREP_BUFS) as wrep_pool,
            tc.tile_pool(name="rcp", bufs=RCP_BUFS) as rcp_pool,
            tc.tile_pool(name="prod", bufs=PROD_BUFS) as prod_pool,
            tc.tile_pool(name="acc", bufs=3) as acc_pool,
            tc.tile_pool(name="psum", bufs=PSUM_BUFS, space="PSUM") as psum_pool,
            tc.tile_pool(name="ppsum", bufs=PPSUM_BUFS, space="PSUM") as ppsum_pool,
        ):
            cvals = sorted({float(-(ui - TR)) for ui in range(TAPS)} | {1.0})
            cmap = {}
            for ci, v in enumerate(cvals):
                ct = const_pool.tile([128, 1], f32, tag=f"c{ci}", name=f"const{ci}")
                nc.vector.memset(ct[:, :], v)
                cmap[v] = ct

            xsb = slab_pool.tile([C + 1, RSLAB, WP], bf16)
            nc.sync.dma_start(xsb[:, :, :], xslab_d.ap()[:, :, :])
            woffb = const_pool.tile([C + 1, KK * 2 * KK], bf16)
            nc.sync.dma_start(woffb[:, :], woffb_d.ap()[:, :])
            wstack = const_pool.tile([C, WTOT], bf16)
            nc.sync.dma_start(wstack[:, :], wstack_d.ap()[:, :])

            import contextlib
            loop_cm = tc.For_i(0, LOOPT, 1) if LOOPT > 1 else contextlib.nullcontext()
            with loop_cm:
              for hh in range(TILE_ROWS):
                r0 = 2 * hh + HALO

                # ---- offset conv, whole row-pair: p_ps[128, (cb, 18)] ----
                p_ps = ppsum_pool.tile([128, CB * 2 * KK], f32, tag="p")
                for cb in range(CB):
                    c0 = PADC + cb * 64
                    for row in range(2):
                        for k in range(KK):
                            by, bx = BASE[k]
                            nc.tensor.matmul(
                                p_ps[row * 64:(row + 1) * 64,
                                     cb * 18:(cb + 1) * 18],
                                xsb[0:C + 1, r0 + row + by,
                                    c0 + bx:c0 + bx + 64],
                                woffb[:, k * 18:(k + 1) * 18],
                                start=(k == 0), stop=(k == KK - 1),
                            )

                # ---- hat weights: WY/WX [128, (cb, k, u)] ----
                wy = wts_pool.tile([128, CB * NW], f32, tag="wy")
                wx = wts_pool.tile([128, CB * NW], f32, tag="wx")
                if MERGE_RELU:
                    # tmp3 layout (cb, ui, k); one wide Relu per axis with a
                    # permuted input AP (cb, k, ui)
                    tmp3 = wts_pool.tile([128, CB * NW], f32, tag="tmp3")
                    for ax, wt in ((0, wy), (1, wx)):
                        for ui in range(TAPS):
                            u = ui - TR
                            nc.scalar.activation(
                                tmp3.rearrange("p (c u k) -> p c u k",
                                               u=TAPS, k=KK)[:, :, ui, :],
                                p_ps.rearrange("p (c t) -> p c t", t=2 * KK)
                                    [:, :, ax * KK:(ax + 1) * KK],
                                AF.Abs, bias=cmap[float(-u)][:, :])
                        nc.scalar.activation(
                            wt.rearrange("p (c k u) -> p c k u",
                                         u=TAPS, k=KK),
                            tmp3.rearrange("p (c u k) -> p c u k",
                                           u=TAPS, k=KK)
                                .transpose([0, 1, 3, 2]),
                            AF.Relu, scale=-1.0, bias=cmap[1.0][:, :])
                else:
                    tmp = wts_pool.tile([128, CB * KK], f32, tag="tmp")
                    for ax, wt in ((0, wy), (1, wx)):
                        for ui in range(TAPS):
                            u = ui - TR
                            nc.scalar.activation(
                                tmp[:, :],
                                p_ps.rearrange("p (c t) -> p c t", t=2 * KK)
                                    [:, :, ax * KK:(ax + 1) * KK],
                                AF.Abs, bias=cmap[float(-u)][:, :])
                            nc.scalar.activation(
                                wt.rearrange("p (c k t) -> p c k t",
                                             t=TAPS, k=KK)[:, :, :, ui],
                                tmp[:, :], AF.Relu,
                                scale=-1.0, bias=cmap[1.0][:, :])
                # ---- wrep [128, (cb, t_global, o)] bf16, per row-pair ----
                wrep = wrep_pool.tile([128, CB * NT * CO], bf16, tag="wrep")
                wr4 = wrep.rearrange("p (c t o) -> p c t o", t=NT, o=CO)
                if GROUP_SEED:
                    # fused wy*wx -> wrep seeds: one TT per (a,b) grid.
                    # wy col of term (k=3a+b, u) = 27c + 8a + 3b + (sy+2);
                    # wx col = 27c + 9a + 2b + (sx+2).
                    from concourse.ap import AP as RawAP
                    wy_t = wy[:, :].tensor
                    wx_t = wx[:, :].tensor
                    wr_t = wrep[:, :].tensor
                    FS = CB * NW          # 81
                    WF = CB * NT * CO     # 15552
                    for sy, sx, a0, b0, na, nb, t0 in GROUP_GRIDS:
                        src_y = RawAP(wy_t, 8 * a0 + 3 * b0 + (sy + 2),
                                      [[FS, 128], [NW, CB],
                                       [8, na], [3, nb]])
                        src_x = RawAP(wx_t, 9 * a0 + 2 * b0 + (sx + 2),
                                      [[FS, 128], [NW, CB],
                                       [9, na], [2, nb]])
                        dst = RawAP(wr_t, t0 * CO,
                                    [[WF, 128], [NT * CO, CB],
                                     [nb * CO, na], [CO, nb]])
                        nc.vector.tensor_tensor(dst, src_y, src_x, MUL)
                else:
                    # wyx in k-major order: col = cb*81 + 9k + 3u + v  (bf16)
                    wyx = wts_pool.tile([128, CB * NT], bf16, tag="wyx")
                    for ui in range(TAPS):
                        for vi in range(TAPS):
                            nc.vector.tensor_tensor(
                                wyx.rearrange("p (c k t) -> p c k t",
                                              t=TT2, k=KK)
                                   [:, :, :, ui * TAPS + vi],
                                wy.rearrange("p (c k t) -> p c k t",
                                             t=TAPS, k=KK)[:, :, :, ui],
                                wx.rearrange("p (c k t) -> p c k t",
                                             t=TAPS, k=KK)[:, :, :, vi],
                                MUL)
                    wv = wyx.rearrange("p (c t) -> p c t", t=NT)
                    for src0, nb, t0 in SEED_ROWS:
                        src = (wv[:, :, src0:src0 + 8 * (nb - 1) + 1:8]
                               if nb > 1 else wv[:, :, src0:src0 + 1])
                        nc.vector.tensor_copy(
                            wr4[:, :, t0:t0 + nb, 0:1], src.unsqueeze(3))
                w = 1
                while w < CO:
                    nc.vector.tensor_copy(
                        wr4[:, :, :, w:2 * w], wr4[:, :, :, 0:w])
                    w *= 2

                for cb in range(CB):
                    t_idx = hh * CB + cb
                    c0 = PADC + cb * 64

                    prod = prod_pool.tile([128, NT * CO], bf16, tag="prod",
                                          name=f"prod_{t_idx}")
                    woff = 0
                    for ci, ch in enumerate(CHUNKS):
                        csz = CHUNK_SIZES[ci]
                        r_ps = psum_pool.tile([128, CHUNK_MAX * CO], f32,
                                              tag="r")
                        coff = 0
                        for g in ch:
                            sy, sx, ks = GROUPS[g]
                            nk = len(ks)
                            for row in range(2):
                                nc.tensor.matmul(
                                    r_ps[row * 64:(row + 1) * 64,
                                         coff:coff + nk * CO],
                                    xsb[0:C, r0 + row + sy,
                                        c0 + sx:c0 + sx + 64],
                                    wstack[:, woff:woff + nk * CO],
                                    start=True, stop=True)
                            coff += nk * CO
                            woff += nk * CO
                        t0 = TERM_BASE[ch[0]]
                        rcp = rcp_pool.tile([128, CHUNK_MAX * CO], bf16,
                                            tag="rcp")
                        nc.scalar.activation(rcp[:, 0:csz * CO],
                                             r_ps[:, 0:csz * CO], AF.Copy)
                        wr0 = (cb * NT + t0) * CO
                        if POOL_PROD and ci == len(CHUNKS) - 1:
                            nc.gpsimd.tensor_tensor(
                                prod[:, t0 * CO:(t0 + csz) * CO],
                                rcp[:, 0:csz * CO],
                                wrep[:, wr0:wr0 + csz * CO], MUL)
                        elif POOL_XTERMS and ci == len(CHUNKS) - 2:
                            cd = csz - POOL_XTERMS
                            nc.vector.tensor_tensor(
                                prod[:, t0 * CO:(t0 + cd) * CO],
                                rcp[:, 0:cd * CO],
                                wrep[:, wr0:wr0 + cd * CO], MUL)
                            nc.gpsimd.tensor_tensor(
                                prod[:, (t0 + cd) * CO:(t0 + csz) * CO],
                                rcp[:, cd * CO:csz * CO],
                                wrep[:, wr0 + cd * CO:wr0 + csz * CO], MUL)
                        else:
                            step = PROD_SPLIT if PROD_SPLIT > 0 else csz
                            for s0 in range(0, csz, step):
                                s1 = min(s0 + step, csz)
                                nc.vector.tensor_tensor(
                                    prod[:, (t0 + s0) * CO:(t0 + s1) * CO],
                                    rcp[:, s0 * CO:s1 * CO],
                                    wrep[:, wr0 + s0 * CO:wr0 + s1 * CO],
                                    MUL)

                    # ---- block tree-fold: 81 blocks of 64 -> 1 ----
                    def tree(eng, b0, nb_, stop=1):
                        # reduce blocks [b0, b0+nb_) into [b0, b0+stop)
                        while nb_ > stop:
                            half = min(nb_ // 2, nb_ - stop)
                            eng.tensor_tensor(
                                prod[:, b0 * CO:(b0 + half) * CO],
                                prod[:, b0 * CO:(b0 + half) * CO],
                                prod[:, (b0 + nb_ - half) * CO:
                                        (b0 + nb_) * CO],
                                ADD)
                            nb_ -= half

                    if POOL_SPLIT > 0:
                        nd = NT - POOL_SPLIT
                        tree(nc.gpsimd, nd, POOL_SPLIT)
                        if F32_TAIL:
                            tree(nc.vector, 0, nd, stop=4)
                            nc.gpsimd.tensor_tensor(
                                prod[:, 3 * CO:4 * CO], prod[:, 3 * CO:4 * CO],
                                prod[:, nd * CO:(nd + 1) * CO], ADD)
                            prodf = acc_pool.tile([128, 2 * CO], f32,
                                                  tag="prodf")
                            nc.gpsimd.tensor_tensor(
                                prodf[:, :], prod[:, 0:2 * CO],
                                prod[:, 2 * CO:4 * CO], ADD)
                            nc.gpsimd.tensor_tensor(
                                prodf[:, 0:CO], prodf[:, 0:CO],
                                prodf[:, CO:2 * CO], ADD)
                            nc.sync.dma_start(
                                out_d.ap()[t_idx * 128:(t_idx + 1) * 128, :],
                                prodf[:, 0:CO])
                            continue
                        tree(nc.vector, 0, nd)
                        nc.vector.tensor_tensor(
                            prod[:, 0:CO], prod[:, 0:CO],
                            prod[:, nd * CO:(nd + 1) * CO], ADD)
                    else:
                        nb_ = NT
                        li = 0
                        while nb_ > 1:
                            half = nb_ // 2
                            eng = nc.gpsimd if li in POOL_LEVELS else nc.vector
                            eng.tensor_tensor(
                                prod[:, 0:half * CO],
                                prod[:, 0:half * CO],
                                prod[:, (nb_ - half) * CO:nb_ * CO],
                                ADD)
                            nb_ -= half
                            li += 1
                    osum = acc_pool.tile([128, CO], f32, tag="osum")
                    if OSUM_ACT:
                        nc.scalar.activation(osum[:, :], prod[:, 0:CO],
                                             AF.Copy)
                    else:
                        nc.vector.tensor_copy(osum[:, :], prod[:, 0:CO])
                    nc.sync.dma_start(
                        out_d.ap()[t_idx * 128:(t_idx + 1) * 128, :],
                        osum[:, :])

    nc.compile()
    return nc


def _prep_weights(w_deform, w_offset, b_offset):
    perm = DYPERM + DXPERM
    wo = w_offset[perm]
    bo = b_offset[perm]
    woffb = np.zeros((C + 1, KK * 18), np.float32)
    for k in range(KK):
        ky, kx = k // 3, k % 3
        woffb[:C, k * 18:(k + 1) * 18] = wo[:, :, ky, kx].T
    woffb[C, 4 * 18:5 * 18] = bo
    blocks = []
    for g in GORDER:
        sy, sx, ks = GROUPS[g]
        for k in ks:
            blocks.append(w_deform[:, :, k // 3, k % 3].T)  # [C, CO]
    wstack = np.concatenate(blocks, axis=1).astype(np.float32)
    import ml_dtypes
    return woffb.astype(ml_dtypes.bfloat16), wstack.astype(ml_dtypes.bfloat16)


def kernel(x, w_deform, w_offset, b_offset):
    from concourse.bass_utils import run_bass_kernel_spmd

    if "nc" not in _CACHE:
        _CACHE["nc"] = _build_program()
    nc = _CACHE["nc"]

    woffb, wstack = _prep_weights(
        np.asarray(w_deform, np.float32),
        np.asarray(w_offset, np.float32),
        np.asarray(b_offset, np.float32))

    x = np.asarray(x, np.float32)
    in_maps = []
    for core in range(N_CORES):
        b, half = core // HALVES, core % HALVES
        import ml_dtypes
        slab = np.zeros((C + 1, RSLAB, WP), ml_dtypes.bfloat16)
        slab[C] = 1.0
        r_lo = half * ROWS - HALO
        r_hi = half * ROWS + ROWS + HALO
        src_lo, src_hi = max(r_lo, 0), min(r_hi, H)
        slab[:C, src_lo - r_lo:src_hi - r_lo, PADC:PADC + W] = \
            x[b, :, src_lo:src_hi, :].astype(ml_dtypes.bfloat16)
        in_maps.append({"xslab": slab, "woffb": woffb, "wstack": wstack})

    res = run_bass_kernel_spmd(nc, in_maps, core_ids=list(range(N_CORES)))

    out = np.empty((B, CO, H, W), np.float32)
    for core in range(N_CORES):
        b, half = core // HALVES, core % HALVES
        o = res.results[core]["out"]          # [144*128, 64]
        o = o.reshape(TILE_ROWS, CB, 2, 64, CO)
        o = o.transpose(4, 0, 2, 1, 3).reshape(CO, ROWS, W)
        out[b, :, half * ROWS:(half + 1) * ROWS, :] = o
    return out


if __name__ == "__main__":
    xs = {k: np.load(f"/tmp/in_{k}.npy") for k in
          ("x", "w_deform", "w_offset", "b_offset")}
    got = kernel(**xs)
    exp = np.load("/tmp/expected.npy")
    err = np.abs(got - exp)
    rel = np.linalg.norm(got - exp) / np.linalg.norm(exp)
    print(f"absmax={err.max():.6f} rel-l2={rel:.3e}")


# revision 5
# speedup vs baseline: 4.2132x; 1.0444x over previous
# Deformable Conv2d (B=4, C=CO=64, H=W=192, K=3, pad=1) on 8 Trainium2 cores.
#
# v2: wide-op redesign of the modulated accumulation (vs the per-term
# scalar_tensor_tensor baseline).  Math: with bilinear offsets |d|<1.5,
#   out[px,o] = sum_{t=(k,u,v)} wy[px,k,u]*wx[px,k,v] * (W_k @ x_s(t))[px,o]
# where s(t) = base_k + (u,v) is one of 25 absolute shifts.  Per 128-pixel
# tile (2 rows x 64 cols):
#   - PE: 26 shift-group matmuls with M=128 (both rows in one matmul via a
#     2D stationary AP), into 4 PSUM chunks of [24,24,24,9] terms.
#   - ACT: one wide PSUM->SBUF bf16 copy per chunk (+ hat-weight builds).
#   - DVE: hat weights replicated x64 into wrep[128, 81*64] bf16 once per
#     row-pair (28 fused wy*wx->seed tensor_tensors over uniform strided
#     grids -- wy col of term (k=3a+b,u) is 27c+8a+3b+(sy+2), wx col is
#     27c+9a+2b+(sx+2) -- + 6 log2 doublings in 4x mode), then 4 wide
#     tensor_tensor products (2x bf16) and a 7-level block tree-fold.
#   - GpSimd is left IDLE on purpose: any Pool load regressed real HW
#     hard (DVE<->GpSimd shared-port serialization), despite sim gains.
import os
import numpy as np

B, C, CO, H, W = 4, 64, 64, 192, 192
K, PAD, KK = 3, 1, 9
N_CORES = 8
HALVES = N_CORES // B            # 2 half-images per batch sample
ROWS = H // HALVES               # 96 rows per core
HALO = 3                         # row halo each side
PADC = 3                         # col zero-pad each side
WP = W + 2 * PADC                # 198
RSLAB = ROWS + 2 * HALO          # 102
TAPS = 3
TR = (TAPS - 1) // 2
CB = 3                           # col blocks of 64 per row-pair
TILE_ROWS = ROWS // 2            # 48 row-pairs
N_TILES = TILE_ROWS * CB         # 144
NT = 81                          # modulation terms per pixel
LOOPT = int(os.environ.get("DFC_LOOPT", "1"))

# fold levels run on GpSimd (comma list of level indices, level 0 = widest)
POOL_LEVELS = {int(x) for x in
               os.environ.get("DFC_POOL_LEVELS", "").split(",") if x != ""}
WTS_BUFS = int(os.environ.get("DFC_WTS_BUFS", "2"))
WREP_BUFS = int(os.environ.get("DFC_WREP_BUFS", "2"))
RCP_BUFS = int(os.environ.get("DFC_RCP_BUFS", "3"))
PROD_BUFS = int(os.environ.get("DFC_PROD_BUFS", "2"))
PSUM_BUFS = int(os.environ.get("DFC_PSUM_BUFS", "2"))
PPSUM_BUFS = int(os.environ.get("DFC_PPSUM_BUFS", "2"))
# parallel fold trees: Pool reduces the last POOL_SPLIT blocks while DVE
# reduces the rest; 0 disables (POOL_LEVELS ping-pong mode instead)
POOL_SPLIT = int(os.environ.get("DFC_POOL_SPLIT", "0"))
POOL_PROD = int(os.environ.get("DFC_POOL_PROD", "0"))   # last chunk's product on Pool
POOL_XTERMS = int(os.environ.get("DFC_POOL_XTERMS", "0"))  # extra product terms on Pool
OSUM_ACT = int(os.environ.get("DFC_OSUM_ACT", "1"))     # final copy on ACT
F32_TAIL = int(os.environ.get("DFC_F32_TAIL", "0"))     # finish fold in f32 on Pool
MERGE_RELU = int(os.environ.get("DFC_MERGE_RELU", "1"))  # 2 wide Relu vs 6
DMA_DOUBLE = int(os.environ.get("DFC_DMA_DOUBLE", "0"))  # widest wrep doubling on DMA
GROUP_SEED = int(os.environ.get("DFC_GROUP_SEED", "1"))  # per-group wy*wx -> wrep seed
PROD_SPLIT = int(os.environ.get("DFC_PROD_SPLIT", "0"))  # split products into <=N-term ops

DYPERM = [0, 4, 8, 12, 16, 3, 7, 11, 15]
DXPERM = [2, 6, 10, 14, 1, 5, 9, 13, 17]

BASE = [(k // 3 - 1, k % 3 - 1) for k in range(KK)]  # (by, bx) per k

SHIFTS = []
for sy in range(-1 - TR, 2 + TR):
    for sx in range(-1 - TR, 2 + TR):
        ks = [k for k in range(KK)
              if abs(sy - BASE[k][0]) <= TR and abs(sx - BASE[k][1]) <= TR]
        if ks:
            SHIFTS.append((sy, sx, ks))
MAX_GROUP_K = 8  # matmul N = nk*64 <= 512 (one PSUM bank)

GROUPS = []
for sy, sx, ks in SHIFTS:
    for i in range(0, len(ks), MAX_GROUP_K):
        GROUPS.append((sy, sx, ks[i:i + MAX_GROUP_K]))

CHUNK_MAX = int(os.environ.get("DFC_CHUNK_MAX", "24"))  # terms per PSUM chunk


def _pack_chunks():
    order = sorted(range(len(GROUPS)), key=lambda g: -len(GROUPS[g][2]))
    chunks, sizes = [], []
    for g in order:
        nk = len(GROUPS[g][2])
        for ci in range(len(chunks)):
            if sizes[ci] + nk <= CHUNK_MAX:
                chunks[ci].append(g)
                sizes[ci] += nk
                break
        else:
            chunks.append([g])
            sizes.append(nk)
    return chunks, sizes


CHUNKS, CHUNK_SIZES = _pack_chunks()
GORDER = [g for ch in CHUNKS for g in ch]
TERM_BASE = {}
_t = 0
for g in GORDER:
    TERM_BASE[g] = _t
    _t += len(GROUPS[g][2])
assert _t == NT
WTOT = NT * CO


def _group_grids():
    """Per-group (sy,sx): full (a,b) grids of its ks for the fused
    wy*wx -> wrep-seed tensor_tensor.  Ragged groups split by a-row."""
    grids = []
    for g in GORDER:
        sy, sx, ks = GROUPS[g]
        t0 = TERM_BASE[g]
        rows = {}
        for k in ks:
            rows.setdefault(k // 3, []).append(k % 3)
        avs = sorted(rows)
        full = (len({tuple(v) for v in rows.values()}) == 1 and
                avs == list(range(avs[0], avs[0] + len(avs))))
        if full:
            bs = rows[avs[0]]
            assert bs == list(range(bs[0], bs[0] + len(bs)))
            grids.append((sy, sx, avs[0], bs[0], len(avs), len(bs), t0))
        else:
            j = 0
            for a in avs:
                bs = rows[a]
                assert bs == list(range(bs[0], bs[0] + len(bs)))
                grids.append((sy, sx, a, bs[0], 1, len(bs), t0 + j))
                j += len(bs)
    return grids


GROUP_GRIDS = _group_grids()


def _seed_rows():
    """Seed-copy plan: per group, per k//3-row, a stride-8 source slice in
    k-major wyx (src col of term (k,u,v) is 9k+3u+v = 8k + (3sy+sx+8))."""
    rows = []
    for g in GORDER:
        sy, sx, ks = GROUPS[g]
        cc = 3 * sy + sx + 8
        t0 = TERM_BASE[g]
        j = 0
        from itertools import groupby
        for a, grp in groupby(ks, key=lambda k: k // 3):
            bs = [k % 3 for k in grp]
            assert bs == list(range(bs[0], bs[0] + len(bs)))
            src0 = 8 * (3 * a + bs[0]) + cc
            rows.append((src0, len(bs), t0 + j))
            j += len(bs)
    return rows


SEED_ROWS = _seed_rows()

_CACHE = {}


def _build_program():
    import concourse.bacc as bacc
    import concourse.mybir as mybir
    from concourse import tile

    f32 = mybir.dt.float32
    bf16 = mybir.dt.bfloat16
    MUL = mybir.AluOpType.mult
    ADD = mybir.AluOpType.add
    AF = mybir.ActivationFunctionType

    nc = bacc.Bacc("TRN2", num_devices=N_CORES)
    xslab_d = nc.dram_tensor("xslab", [C + 1, RSLAB, WP], bf16, kind="ExternalInput")
    woffb_d = nc.dram_tensor("woffb", [C + 1, KK * 2 * KK], bf16, kind="ExternalInput")
    wstack_d = nc.dram_tensor("wstack", [C, WTOT], bf16, kind="ExternalInput")
    out_d = nc.dram_tensor("out", [N_TILES * 128, CO], f32, kind="ExternalOutput")

    NW = KK * TAPS               # 27
    TT2 = TAPS * TAPS            # 9

    with tile.TileContext(nc) as tc:
        with (
            tc.tile_pool(name="slab", bufs=1) as slab_pool,
            tc.tile_pool(name="consts", bufs=1) as const_pool,
            tc.tile_pool(name="wts", bufs=WTS_BUFS) as wts_pool,
            tc.tile_pool(name="wrep", bufs=WREP_BUFS) as wrep_pool,
            tc.tile_pool(name="rcp", bufs=RCP_BUFS) as rcp_pool,
            tc.tile_pool(name="prod", bufs=PROD_BUFS) as prod_pool,
            tc.tile_pool(name="acc", bufs=3) as acc_pool,
            tc.tile_pool(name="psum", bufs=PSUM_BUFS, space="PSUM") as psum_pool,
            tc.tile_pool(name="ppsum", bufs=PPSUM_BUFS, space="PSUM") as ppsum_pool,
        ):
            cvals = sorted({float(-(ui - TR)) for ui in range(TAPS)} | {1.0})
            cmap = {}
            for ci, v in enumerate(cvals):
                ct = const_pool.tile([128, 1], f32, tag=f"c{ci}", name=f"const{ci}")
                nc.vector.memset(ct[:, :], v)
                cmap[v] = ct

            xsb = slab_pool.tile([C + 1, RSLAB, WP], bf16)
            nc.sync.dma_start(xsb[:, :, :], xslab_d.ap()[:, :, :])
            woffb = const_pool.tile([C + 1, KK * 2 * KK], bf16)
            nc.sync.dma_start(woffb[:, :], woffb_d.ap()[:, :])
            wstack = const_pool.tile([C, WTOT], bf16)
            nc.sync.dma_start(wstack[:, :], wstack_d.ap()[:, :])

            import contextlib
            loop_cm = tc.For_i(0, LOOPT, 1) if LOOPT > 1 else contextlib.nullcontext()
            with loop_cm:
              for hh in range(TILE_ROWS):
                r0 = 2 * hh + HALO

                # ---- offset conv, whole row-pair: p_ps[128, (cb, 18)] ----
                p_ps = ppsum_pool.tile([128, CB * 2 * KK], f32, tag="p")
                for cb in range(CB):
                    c0 = PADC + cb * 64
                    for row in range(2):
                        for k in range(KK):
                            by, bx = BASE[k]
                            nc.tensor.matmul(
                                p_ps[row * 64:(row + 1) * 64,
                                     cb * 18:(cb + 1) * 18],
                                xsb[0:C + 1, r0 + row + by,
                                    c0 + bx:c0 + bx + 64],
                                woffb[:, k * 18:(k + 1) * 18],
                                start=(k == 0), stop=(k == KK - 1),
                            )

                # ---- hat weights: WY/WX [128, (cb, k, u)] ----
                wy = wts_pool.tile([128, CB * NW], f32, tag="wy")
                wx = wts_pool.tile([128, CB * NW], f32, tag="wx")
                if MERGE_RELU:
                    # tmp3 layout (cb, ui, k); one wide Relu per axis with a
                    # permuted input AP (cb, k, ui)
                    tmp3 = wts_pool.tile([128, CB * NW], f32, tag="tmp3")
                    for ax, wt in ((0, wy), (1, wx)):
                        for ui in range(TAPS):
                            u = ui - TR
                            nc.scalar.activation(
                                tmp3.rearrange("p (c u k) -> p c u k",
                                               u=TAPS, k=KK)[:, :, ui, :],
                                p_ps.rearrange("p (c t) -> p c t", t=2 * KK)
                                    [:, :, ax * KK:(ax + 1) * KK],
                                AF.Abs, bias=cmap[float(-u)][:, :])
                        nc.scalar.activation(
                            wt.rearrange("p (c k u) -> p c k u",
                                         u=TAPS, k=KK),
                            tmp3.rearrange("p (c u k) -> p c u k",
                                           u=TAPS, k=KK)
                                .transpose([0, 1, 3, 2]),
                            AF.Relu, scale=-1.0, bias=cmap[1.0][:, :])
                else:
                    tmp = wts_pool.tile([128, CB * KK], f32, tag="tmp")
                    for ax, wt in ((0, wy), (1, wx)):
                        for ui in range(TAPS):
                            u = ui - TR
                            nc.scalar.activation(
                                tmp[:, :],
                                p_ps.rearrange("p (c t) -> p c t", t=2 * KK)
                                    [:, :, ax * KK:(ax + 1) * KK],
                                AF.Abs, bias=cmap[float(-u)][:, :])
                            nc.scalar.activation(
                                wt.rearrange("p (c k t) -> p c k t",
                                             t=TAPS, k=KK)[:, :, :, ui],
                                tmp[:, :], AF.Relu,
                                scale=-1.0, bias=cmap[1.0][:, :])
                # ---- wrep [128, (cb, t_global, o)] bf16, per row-pair ----
                wrep = wrep_pool.tile([128, CB * NT * CO], bf16, tag="wrep")
                wr4 = wrep.rearrange("p (c t o) -> p c t o", t=NT, o=CO)
                if GROUP_SEED:
                    # fused wy*wx -> wrep seeds: one TT per (a,b) grid.
                    # wy col of term (k=3a+b, u) = 27c + 8a + 3b + (sy+2);
                    # wx col = 27c + 9a + 2b + (sx+2).
                    from concourse.ap import AP as RawAP
                    wy_t = wy[:, :].tensor
                    wx_t = wx[:, :].tensor
                    wr_t = wrep[:, :].tensor
                    FS = CB * NW          # 81
                    WF = CB * NT * CO     # 15552
                    for sy, sx, a0, b0, na, nb, t0 in GROUP_GRIDS:
                        src_y = RawAP(wy_t, 8 * a0 + 3 * b0 + (sy + 2),
                                      [[FS, 128], [NW, CB],
                                       [8, na], [3, nb]])
                        src_x = RawAP(wx_t, 9 * a0 + 2 * b0 + (sx + 2),
                                      [[FS, 128], [NW, CB],
                                       [9, na], [2, nb]])
                        dst = RawAP(wr_t, t0 * CO,
                                    [[WF, 128], [NT * CO, CB],
                                     [nb * CO, na], [CO, nb]])
                        nc.vector.tensor_tensor(dst, src_y, src_x, MUL)
                else:
                    # wyx in k-major order: col = cb*81 + 9k + 3u + v  (bf16)
                    wyx = wts_pool.tile([128, CB * NT], bf16, tag="wyx")
                    for ui in range(TAPS):
                        for vi in range(TAPS):
                            nc.vector.tensor_tensor(
                                wyx.rearrange("p (c k t) -> p c k t",
                                              t=TT2, k=KK)
                                   [:, :, :, ui * TAPS + vi],
                                wy.rearrange("p (c k t) -> p c k t",
                                             t=TAPS, k=KK)[:, :, :, ui],
                                wx.rearrange("p (c k t) -> p c k t",
                                             t=TAPS, k=KK)[:, :, :, vi],
                                MUL)
                    wv = wyx.rearrange("p (c t) -> p c t", t=NT)
                    for src0, nb, t0 in SEED_ROWS:
                        src = (wv[:, :, src0:src0 + 8 * (nb - 1) + 1:8]
                               if nb > 1 else wv[:, :, src0:src0 + 1])
                        nc.vector.tensor_copy(
                            wr4[:, :, t0:t0 + nb, 0:1], src.unsqueeze(3))
                w = 1
                while w < CO:
                    nc.vector.tensor_copy(
                        wr4[:, :, :, w:2 * w], wr4[:, :, :, 0:w])
                    w *= 2

                for cb in range(CB):
                    t_idx = hh * CB + cb
                    c0 = PADC + cb * 64

                    prod = prod_pool.tile([128, NT * CO], bf16, tag="prod",
                                          name=f"prod_{t_idx}")
                    woff = 0
                    for ci, ch in enumerate(CHUNKS):
                        csz = CHUNK_SIZES[ci]
                        r_ps = psum_pool.tile([128, CHUNK_MAX * CO], f32,
                                              tag="r")
                        coff = 0
                        for g in ch:
                            sy, sx, ks = GROUPS[g]
                            nk = len(ks)
                            for row in range(2):
                                nc.tensor.matmul(
                                    r_ps[row * 64:(row + 1) * 64,
                                         coff:coff + nk * CO],
                                    xsb[0:C, r0 + row + sy,
                                        c0 + sx:c0 + sx + 64],
                                    wstack[:, woff:woff + nk * CO],
                                    start=True, stop=True)
                            coff += nk * CO
                            woff += nk * CO
                        t0 = TERM_BASE[ch[0]]
                        rcp = rcp_pool.tile([128, CHUNK_MAX * CO], bf16,
                                            tag="rcp")
                        nc.scalar.activation(rcp[:, 0:csz * CO],
                                             r_ps[:, 0:csz * CO], AF.Copy)
                        wr0 = (cb * NT + t0) * CO
                        if POOL_PROD and ci == len(CHUNKS) - 1:
                            nc.gpsimd.tensor_tensor(
                                prod[:, t0 * CO:(t0 + csz) * CO],
                                rcp[:, 0:csz * CO],
                                wrep[:, wr0:wr0 + csz * CO], MUL)
                        elif POOL_XTERMS and ci == len(CHUNKS) - 2:
                            cd = csz - POOL_XTERMS
                            nc.vector.tensor_tensor(
                                prod[:, t0 * CO:(t0 + cd) * CO],
                                rcp[:, 0:cd * CO],
                                wrep[:, wr0:wr0 + cd * CO], MUL)
                            nc.gpsimd.tensor_tensor(
                                prod[:, (t0 + cd) * CO:(t0 + csz) * CO],
                                rcp[:, cd * CO:csz * CO],
                                wrep[:, wr0 + cd * CO:wr0 + csz * CO], MUL)
                        else:
                            step = PROD_SPLIT if PROD_SPLIT > 0 else csz
                            for s0 in range(0, csz, step):
                                s1 = min(s0 + step, csz)
                                nc.vector.tensor_tensor(
                                    prod[:, (t0 + s0) * CO:(t0 + s1) * CO],
                                    rcp[:, s0 * CO:s1 * CO],
                                    wrep[:, wr0 + s0 * CO:wr0 + s1 * CO],
                                    MUL)

                    # ---- block tree-fold: 81 blocks of 64 -> 1 ----
                    def tree(eng, b0, nb_, stop=1):
                        # reduce blocks [b0, b0+nb_) into [b0, b0+stop)
                        while nb_ > stop:
                            half = min(nb_ // 2, nb_ - stop)
                            eng.tensor_tensor(
                                prod[:, b0 * CO:(b0 + half) * CO],
                                prod[:, b0 * CO:(b0 + half) * CO],
                                prod[:, (b0 + nb_ - half) * CO:
                                        (b0 + nb_) * CO],
                                ADD)
                            nb_ -= half

                    if POOL_SPLIT > 0:
                        nd = NT - POOL_SPLIT
                        tree(nc.gpsimd, nd, POOL_SPLIT)
                        if F32_TAIL:
                            tree(nc.vector, 0, nd, stop=4)
                            nc.gpsimd.tensor_tensor(
                                prod[:, 3 * CO:4 * CO], prod[:, 3 * CO:4 * CO],
                                prod[:, nd * CO:(nd + 1) * CO], ADD)
                            prodf = acc_pool.tile([128, 2 * CO], f32,
                                                  tag="prodf")
                            nc.gpsimd.tensor_tensor(
                                prodf[:, :], prod[:, 0:2 * CO],
                                prod[:, 2 * CO:4 * CO], ADD)
                            nc.gpsimd.tensor_tensor(
                                prodf[:, 0:CO], prodf[:, 0:CO],
                                prodf[:, CO:2 * CO], ADD)
                            nc.sync.dma_start(
                                out_d.ap()[t_idx * 128:(t_idx + 1) * 128, :],
                                prodf[:, 0:CO])
                            continue
                        tree(nc.vector, 0, nd)
                        nc.vector.tensor_tensor(
                            prod[:, 0:CO], prod[:, 0:CO],
                            prod[:, nd * CO:(nd + 1) * CO], ADD)
                    else:
                        nb_ = NT
                        li = 0
                        while nb_ > 1:
                            half = nb_ // 2
                            eng = nc.gpsimd if li in POOL_LEVELS else nc.vector
                            eng.tensor_tensor(
                                prod[:, 0:half * CO],
                                prod[:, 0:half * CO],
                                prod[:, (nb_ - half) * CO:nb_ * CO],
                                ADD)
                            nb_ -= half
                            li += 1
                    osum = acc_pool.tile([128, CO], f32, tag="osum")
                    if OSUM_ACT:
                        nc.scalar.activation(osum[:, :], prod[:, 0:CO],
                                             AF.Copy)
                    else:
                        nc.vector.tensor_copy(osum[:, :], prod[:, 0:CO])
                    nc.sync.dma_start(
                        out_d.ap()[t_idx * 128:(t_idx + 1) * 128, :],
                        osum[:, :])

    nc.compile()
    return nc


def _prep_weights(w_deform, w_offset, b_offset):
    perm = DYPERM + DXPERM
    wo = w_offset[perm]
    bo = b_offset[perm]
    woffb = np.zeros((C + 1, KK * 18), np.float32)
    for k in range(KK):
        ky, kx = k // 3, k % 3
        woffb[:C, k * 18:(k + 1) * 18] = wo[:, :, ky, kx].T
    woffb[C, 4 * 18:5 * 18] = bo
    blocks = []
    for g in GORDER:
        sy, sx, ks = GROUPS[g]
        for k in ks:
            blocks.append(w_deform[:, :, k // 3, k % 3].T)  # [C, CO]
    wstack = np.concatenate(blocks, axis=1).astype(np.float32)
    import ml_dtypes
    return woffb.astype(ml_dtypes.bfloat16), wstack.astype(ml_dtypes.bfloat16)


def kernel(x, w_deform, w_offset, b_offset):
    from concourse.bass_utils import run_bass_kernel_spmd

    if "nc" not in _CACHE:
        _CACHE["nc"] = _build_program()
    nc = _CACHE["nc"]

    woffb, wstack = _prep_weights(
        np.asarray(w_deform, np.float32),
        np.asarray(w_offset, np.float32),
        np.asarray(b_offset, np.float32))

    x = np.asarray(x, np.float32)
    in_maps = []
    for core in range(N_CORES):
        b, half = core // HALVES, core % HALVES
        import ml_dtypes
        slab = np.zeros((C + 1, RSLAB, WP), ml_dtypes.bfloat16)
        slab[C] = 1.0
        r_lo = half * ROWS - HALO
        r_hi = half * ROWS + ROWS + HALO
        src_lo, src_hi = max(r_lo, 0), min(r_hi, H)
        slab[:C, src_lo - r_lo:src_hi - r_lo, PADC:PADC + W] = \
            x[b, :, src_lo:src_hi, :].astype(ml_dtypes.bfloat16)
        in_maps.append({"xslab": slab, "woffb": woffb, "wstack": wstack})

    res = run_bass_kernel_spmd(nc, in_maps, core_ids=list(range(N_CORES)))

    out = np.empty((B, CO, H, W), np.float32)
    for core in range(N_CORES):
        b, half = core // HALVES, core % HALVES
        o = res.results[core]["out"]          # [144*128, 64]
        o = o.reshape(TILE_ROWS, CB, 2, 64, CO)
        o = o.transpose(4, 0, 2, 1, 3).reshape(CO, ROWS, W)
        out[b, :, half * ROWS:(half + 1) * ROWS, :] = o
    return out


if __name__ == "__main__":
    xs = {k: np.load(f"/tmp/in_{k}.npy") for k in
          ("x", "w_deform", "w_offset", "b_offset")}
    got = kernel(**xs)
    exp = np.load("/tmp/expected.npy")
    err = np.abs(got - exp)
    rel = np.linalg.norm(got - exp) / np.linalg.norm(exp)
    print(f"absmax={err.max():.6f} rel-l2={rel:.3e}")
